# revision 10
# baseline (speedup 1.0000x reference)
"""Trainium2 Bass kernel for nn_Artificial_label_loss (retrieval_knn).

Shards across 8 NeuronCores: core c handles batch b=c//4 and query chunk
q=c%4 (2048 queries). One brute-force pass computes the (2048 x 8192) L1
distance tile set; row-mins give cham_x, row argmins come from max_index
(value search), and column-mins (for cham_y) are extracted from the same
distance tiles via TensorEngine transposes + free-dim reductions, then
min-combined across the 4 cores of the batch group with a ReduceScatter
that lands exactly this core's cham_y chunk. The epilogue (flow-vs-rigid
select, gather, grid scatter, cross-entropy partial sums) runs on-device
with an AllGather of (cell, label); the host combines two scalar sums.
"""
import os
import numpy as np

from concourse import bass, tile, mybir, bacc
from concourse.bass_utils import run_bass_kernel_spmd
from concourse.masks import make_identity

dt = mybir.dt
Alu = mybir.AluOpType
Act = mybir.ActivationFunctionType
AX = mybir.AxisListType

B, N, M, G = 2, 8192, 8192, 256
X_MIN = -35.0
CELL = abs(2.0 * X_MIN / G)          # 0.2734375, exact in f32
INV_CELL = np.float32(1.0) / np.float32(CELL)

P = 128          # partitions
NQT = 16         # query tiles per core (16*128 = 2048)
CH = 2048        # per-core chunk size
MT = 2048        # M tile size
NMT = M // MT    # 4
NBLK = MT // P   # 16 transpose blocks per M tile
GRP = 8          # transpose blocks per PSUM reduction group

NCORES = 8
RGROUPS = [[0, 1, 2, 3], [4, 5, 6, 7]]


def _build():
    nc = bacc.Bacc("TRN2", target_bir_lowering=False, debug=False,
                   num_devices=NCORES)

    # ---- inputs (per-core shards prepared by host) ----
    pjT = nc.dram_tensor("pjT", [3, M], dt.float32, kind="ExternalInput")
    piqT = nc.dram_tensor("piqT", [3, CH], dt.float32, kind="ExternalInput")
    pj = nc.dram_tensor("pj", [M, 3], dt.float32, kind="ExternalInput")
    flow = nc.dram_tensor("flow", [P, NQT], dt.float32, kind="ExternalInput")
    nf = nc.dram_tensor("nf", [P, NQT], dt.int32, kind="ExternalInput")
    mos0 = nc.dram_tensor("mos0", [P, 512], dt.float32, kind="ExternalInput")
    mos1 = nc.dram_tensor("mos1", [P, 512], dt.float32, kind="ExternalInput")

    o_sums = nc.dram_tensor("o_sums", [P, 2], dt.float32, kind="ExternalOutput")
    o_chamx = nc.dram_tensor("o_chamx", [P, NQT], dt.float32, kind="ExternalOutput")
    o_chamy = nc.dram_tensor("o_chamy", [P, NQT], dt.float32, kind="ExternalOutput")
    o_jstar = nc.dram_tensor("o_jstar", [P, NQT], dt.float32, kind="ExternalOutput")

    def bcast_ap(dram_t, coord, lo, n):
        return bass.AP(tensor=dram_t[:].tensor, offset=coord * dram_t.shape[1] + lo,
                       ap=[[0, P], [1, n]])

    with tile.TileContext(nc) as tc:
        with tc.tile_pool(name="persist", bufs=1) as pp:
            chamx = pp.tile([P, NQT], dt.float32)
            chamy = pp.tile([P, NQT], dt.float32)
            jstar = pp.tile([P, NQT], dt.float32)
            ident = pp.tile([P, P], dt.float32)
            make_identity(nc, ident[:])
            colmin = pp.tile([P, M // P], dt.float32)       # [128, 64]
            nc.vector.memset(colmin[:], 3.0e38)
            consts_i = pp.tile([P, NMT], dt.int32)
            consts = pp.tile([P, NMT], dt.float32)
            nc.gpsimd.iota(consts_i[:], pattern=[[MT, NMT]], base=0,
                           channel_multiplier=0)             # 0,2048,4096,6144
            nc.vector.tensor_copy(consts[:], consts_i[:])

            # ---------------- distance pass ----------------
            with tc.tile_pool(name="p1c", bufs=1) as cp, \
                 tc.tile_pool(name="p1d", bufs=7) as dp, \
                 tc.tile_pool(name="p1t", bufs=2) as tp, \
                 tc.tile_pool(name="p1s", bufs=2) as sp, \
                 tc.tile_pool(name="p1i", bufs=6) as ip, \
                 tc.tile_pool(name="psum", bufs=4, space="PSUM") as psp:
                tgt = []
                for c in range(3):
                    row = []
                    for m in range(NMT):
                        t = cp.tile([P, MT], dt.float32, name=f"tj{c}_{m}")
                        nc.sync.dma_start(t[:], bcast_ap(pjT, c, m * MT, MT))
                        row.append(t)
                    tgt.append(row)

                negq_all = cp.tile([P, NQT, 3], dt.float32)
                for c in range(3):
                    nc.sync.dma_start(
                        bass.AP(tensor=negq_all[:].tensor,
                                offset=negq_all[:].offset + c,
                                ap=[[NQT * 3, P], [3, NQT]]),
                        bass.AP(tensor=piqT[:].tensor, offset=c * CH,
                                ap=[[1, P], [P, NQT]]))
                nc.vector.tensor_scalar(negq_all[:], negq_all[:], -1.0, None,
                                        Alu.mult)

                BIG = 1.0e7
                for k in range(NQT):
                    negq = negq_all[:, k]
                    minacc = sp.tile([P, NMT], dt.float32, tag="minacc")
                    jg = sp.tile([P, NMT], dt.float32, tag="jg")
                    for m in range(NMT):
                        dm = dp.tile([P, MT], dt.float32, tag="d", name=f"d_{k}_{m}")
                        dx = tp.tile([P, MT], dt.float32, tag="dx")
                        dy = tp.tile([P, MT], dt.float32, tag="dy")
                        nc.scalar.activation(dm[:], tgt[2][m][:], Act.Abs,
                                             bias=negq[:, 2:3], scale=1.0)
                        nc.scalar.activation(dx[:], tgt[0][m][:], Act.Abs,
                                             bias=negq[:, 0:1], scale=1.0)
                        nc.scalar.activation(dy[:], tgt[1][m][:], Act.Abs,
                                             bias=negq[:, 1:2], scale=1.0)
                        nc.vector.tensor_tensor(out=dx[:], in0=dx[:], in1=dy[:],
                                                op=Alu.add)
                        nc.gpsimd.tensor_tensor(out=dm[:], in0=dx[:], in1=dm[:],
                                                op=Alu.add)
                        nc.vector.tensor_reduce(minacc[:, m:m + 1], dm[:],
                                                axis=AX.X, op=Alu.min)
                        # per-tile argmin: search this tile's min in this tile
                        minv8 = sp.tile([P, 8], dt.float32, tag="minv8")
                        nc.vector.tensor_copy(
                            minv8[:], minacc[:, m:m + 1].to_broadcast([P, 8]))
                        idx8 = ip.tile([P, 8], dt.uint32, tag="idx8")
                        nc.vector.max_index(idx8[:], minv8[:], dm[:])
                        nc.vector.tensor_copy(jg[:, m:m + 1], idx8[:, 0:1])
                        # column mins via PE transpose + PSUM reduction
                        for g in range(NBLK // GRP):
                            ps = psp.tile([P, GRP * P], dt.float32, tag="ps")
                            for blk in range(GRP):
                                j0 = (g * GRP + blk) * P
                                nc.tensor.transpose(
                                    out=ps[:, blk * P:(blk + 1) * P],
                                    in_=dm[:, j0:j0 + P], identity=ident[:])
                            cm8 = sp.tile([P, GRP], dt.float32, tag="cm8")
                            nc.vector.tensor_reduce(
                                cm8[:], ps[:].rearrange("p (b j) -> p b j", b=GRP),
                                axis=AX.X, op=Alu.min)
                            csl = colmin[:, m * NBLK + g * GRP:
                                         m * NBLK + (g + 1) * GRP]
                            nc.vector.tensor_tensor(out=csl, in0=csl, in1=cm8[:],
                                                    op=Alu.min)
                    nc.vector.tensor_reduce(chamx[:, k:k + 1], minacc[:],
                                            axis=AX.X, op=Alu.min)
                    # pick first tile whose min equals the global min
                    eqm = sp.tile([P, NMT], dt.float32, tag="eqm")
                    nc.vector.tensor_scalar(eqm[:], minacc[:], chamx[:, k:k + 1],
                                            None, Alu.is_equal)
                    pen = sp.tile([P, NMT], dt.float32, tag="pen")
                    nc.vector.tensor_scalar(pen[:], eqm[:], -BIG, BIG, Alu.mult,
                                            Alu.add)
                    nc.vector.tensor_tensor(out=jg[:], in0=jg[:], in1=consts[:],
                                            op=Alu.add)
                    nc.vector.tensor_tensor(out=jg[:], in0=jg[:], in1=pen[:],
                                            op=Alu.add)
                    nc.vector.tensor_reduce(jstar[:, k:k + 1], jg[:],
                                            axis=AX.X, op=Alu.min)

            # ---------------- cham_y via ReduceScatter(min) ----------------
            with tc.tile_pool(name="ep", bufs=1) as ep, \
                 tc.tile_pool(name="epd", bufs=1, space="DRAM") as epd:
                rs_in = epd.tile([M // P, P], dt.float32)     # [64, 128]
                rs_out = epd.tile([M // P // 4, P], dt.float32)  # [16, 128]
                nc.sync.dma_start(
                    bass.AP(tensor=rs_in[:].tensor, offset=rs_in[:].offset,
                            ap=[[1, P], [P, M // P]]), colmin[:])
                nc.gpsimd.collective_compute(
                    "ReduceScatter", Alu.min, replica_groups=RGROUPS,
                    ins=[rs_in[:].opt()], outs=[rs_out[:].opt()])
                nc.sync.dma_start(
                    chamy[:],
                    bass.AP(tensor=rs_out[:].tensor, offset=rs_out[:].offset,
                            ap=[[1, P], [P, NQT]]))

                # ---------------- epilogue ----------------
                nc.sync.dma_start(o_chamx[:], chamx[:])
                nc.sync.dma_start(o_chamy[:], chamy[:])
                nc.sync.dma_start(o_jstar[:], jstar[:])

                flw = ep.tile([P, NQT], dt.float32)
                nc.sync.dma_start(flw[:], flow[:])
                nff = ep.tile([P, NQT], dt.int32)
                nc.sync.dma_start(nff[:], nf[:])
                nff_f = ep.tile([P, NQT], dt.float32)
                nc.vector.tensor_copy(nff_f[:], nff[:])

                rigid = ep.tile([P, NQT], dt.float32)
                nc.vector.tensor_tensor(out=rigid[:], in0=chamx[:], in1=chamy[:],
                                        op=Alu.add)
                nc.vector.tensor_scalar(rigid[:], rigid[:], 0.5, None, Alu.mult)
                dyn = ep.tile([P, NQT], dt.float32)
                nc.vector.tensor_tensor(out=dyn[:], in0=flw[:], in1=rigid[:],
                                        op=Alu.is_gt)
                labels = ep.tile([P, NQT], dt.int32)
                nc.vector.tensor_copy(labels[:], dyn[:])

                # idx = jstar + dyn * (nf - jstar)
                idxf = ep.tile([P, NQT], dt.float32)
                nc.vector.tensor_tensor(out=idxf[:], in0=nff_f[:], in1=jstar[:],
                                        op=Alu.subtract)
                nc.vector.tensor_tensor(out=idxf[:], in0=idxf[:], in1=dyn[:],
                                        op=Alu.mult)
                nc.vector.tensor_tensor(out=idxf[:], in0=idxf[:], in1=jstar[:],
                                        op=Alu.add)
                idxi = ep.tile([P, NQT], dt.int32)
                nc.vector.tensor_copy(idxi[:], idxf[:])

                gxyz = ep.tile([P, NQT, 3], dt.float32)
                for k in range(NQT):
                    nc.gpsimd.indirect_dma_start(
                        out=gxyz[:, k, :], out_offset=None, in_=pj[:],
                        in_offset=bass.IndirectOffsetOnAxis(ap=idxi[:, k:k + 1],
                                                            axis=0))

                # cell indices (neuron-backend astype rounds to nearest)
                cellx = ep.tile([P, NQT], dt.float32)
                celly = ep.tile([P, NQT], dt.float32)
                nc.vector.tensor_scalar(cellx[:], gxyz[:, :, 0], -X_MIN,
                                        float(INV_CELL), Alu.add, Alu.mult)
                nc.vector.tensor_scalar(celly[:], gxyz[:, :, 1], -X_MIN,
                                        float(INV_CELL), Alu.add, Alu.mult)
                cxi = ep.tile([P, NQT], dt.int32)
                cyi = ep.tile([P, NQT], dt.int32)
                nc.vector.tensor_copy(cxi[:], cellx[:])
                nc.vector.tensor_copy(cyi[:], celly[:])
                cells = ep.tile([P, NQT], dt.int32)
                nc.vector.tensor_scalar(cells[:], cxi[:], G, None, Alu.mult)
                nc.vector.tensor_tensor(out=cells[:], in0=cells[:], in1=cyi[:],
                                        op=Alu.add)

                # per-core grid scatter of OWN chunk (query order = last wins),
                # then AllGather the 4 partial grids and merge by chunk priority
                grid_d = epd.tile([G * G, 1], dt.int32)
                initm = ep.tile([P, 512], dt.int32)
                nc.vector.memset(initm[:], -1)
                nc.sync.dma_start(
                    bass.AP(tensor=grid_d[:].tensor, offset=grid_d[:].offset,
                            ap=[[512, P], [1, 512]]), initm[:])
                for col in range(NQT):
                    nc.gpsimd.indirect_dma_start(
                        out=grid_d[:],
                        out_offset=bass.IndirectOffsetOnAxis(
                            ap=cells[:, col:col + 1], axis=0),
                        in_=labels[:, col:col + 1], in_offset=None)
                gag = epd.tile([4, G * G], dt.int32)
                nc.gpsimd.collective_compute(
                    "AllGather", Alu.bypass, replica_groups=RGROUPS,
                    ins=[bass.AP(tensor=grid_d[:].tensor,
                                 offset=grid_d[:].offset,
                                 ap=[[G * G, 1], [1, G * G]]).opt()],
                    outs=[gag[:].opt()])
                # merge: start from chunk 3, fill -1 holes from earlier chunks
                grid = ep.tile([P, 512], dt.int32)
                gtmp = ep.tile([P, 512], dt.int32)
                hole = ep.tile([P, 512], dt.int32)
                for gI in range(3, -1, -1):
                    gap = bass.AP(tensor=gag[:].tensor,
                                  offset=gag[:].offset + gI * G * G,
                                  ap=[[512, P], [1, 512]])
                    if gI == 3:
                        nc.sync.dma_start(grid[:], gap)
                    else:
                        nc.sync.dma_start(gtmp[:], gap)
                        nc.vector.tensor_scalar(hole[:], grid[:], 0.0, None,
                                                Alu.is_lt)
                        nc.vector.copy_predicated(grid[:], hole[:], gtmp[:])

                # CE partial sums
                m0 = ep.tile([P, 512], dt.float32)
                m1 = ep.tile([P, 512], dt.float32)
                nc.sync.dma_start(m0[:], mos0[:])
                nc.sync.dma_start(m1[:], mos1[:])
                e0 = ep.tile([P, 512], dt.float32)
                e1 = ep.tile([P, 512], dt.float32)
                nc.scalar.activation(e0[:], m0[:], Act.Exp)
                nc.scalar.activation(e1[:], m1[:], Act.Exp)
                nc.vector.tensor_tensor(out=e0[:], in0=e0[:], in1=e1[:], op=Alu.add)
                lse = ep.tile([P, 512], dt.float32)
                nc.scalar.activation(lse[:], e0[:], Act.Ln)
                lp0 = ep.tile([P, 512], dt.float32)
                lp1 = ep.tile([P, 512], dt.float32)
                nc.vector.tensor_tensor(out=lp0[:], in0=m0[:], in1=lse[:],
                                        op=Alu.subtract)
                nc.vector.tensor_tensor(out=lp1[:], in0=m1[:], in1=lse[:],
                                        op=Alu.subtract)
                valid = ep.tile([P, 512], dt.float32)
                nc.vector.tensor_scalar(valid[:], grid[:], 0.0, None, Alu.is_ge)
                tsel = ep.tile([P, 512], dt.float32)
                nc.vector.tensor_scalar(tsel[:], grid[:], 0.0, None, Alu.max)
                nc.vector.tensor_tensor(out=lp1[:], in0=lp1[:], in1=lp0[:],
                                        op=Alu.subtract)
                nc.vector.tensor_tensor(out=lp1[:], in0=lp1[:], in1=tsel[:],
                                        op=Alu.mult)
                nc.vector.tensor_tensor(out=lp1[:], in0=lp1[:], in1=lp0[:],
                                        op=Alu.add)
                nc.vector.tensor_tensor(out=lp1[:], in0=lp1[:], in1=valid[:],
                                        op=Alu.mult)
                sums = ep.tile([P, 2], dt.float32)
                nc.vector.tensor_reduce(sums[:, 0:1], lp1[:], axis=AX.X,
                                        op=Alu.add)
                nc.vector.tensor_reduce(sums[:, 1:2], valid[:], axis=AX.X,
                                        op=Alu.add)
                nc.sync.dma_start(o_sums[:], sums[:])

    nc.compile()
    return nc


_NC = None


def _get_nc():
    global _NC
    if _NC is None:
        _NC = _build()
    return _NC


_LAST_RESULTS = None


def kernel(p_i, mos, p_j, error_p_i_flow, nearest_flow):
    global _LAST_RESULTS
    p_i = np.ascontiguousarray(np.asarray(p_i, np.float32))
    p_j = np.ascontiguousarray(np.asarray(p_j, np.float32))
    mos = np.asarray(mos, np.float32)
    flow = np.asarray(error_p_i_flow, np.float32)
    nf = np.asarray(nearest_flow).astype(np.int32)

    nc = _get_nc()
    in_maps = []
    for c in range(NCORES):
        b, q = divmod(c, 4)
        s = q * CH
        in_maps.append({
            "pjT": np.ascontiguousarray(p_j[b].T),
            "piqT": np.ascontiguousarray(p_i[b, s:s + CH].T),
            "pj": p_j[b],
            "flow": np.ascontiguousarray(flow[b, s:s + CH].reshape(NQT, P).T),
            "nf": np.ascontiguousarray(nf[b, s:s + CH, 0].reshape(NQT, P).T),
            "mos0": np.ascontiguousarray(mos[b, 0].reshape(P, 512)),
            "mos1": np.ascontiguousarray(mos[b, 1].reshape(P, 512)),
        })

    trace = bool(int(os.environ.get("KNN_TRACE", "0")))
    tmpdir = os.environ.get("KNN_TMPDIR") or None
    res = run_bass_kernel_spmd(nc, in_maps, core_ids=list(range(NCORES)),
                               trace=trace, tmpdir=tmpdir)
    _LAST_RESULTS = res

    s0 = res.results[0]["o_sums"].astype(np.float64)
    s1 = res.results[4]["o_sums"].astype(np.float64)
    num = np.float32(s0[:, 0].sum() + s1[:, 0].sum())
    den = np.float32(s0[:, 1].sum() + s1[:, 1].sum())
    loss = np.float32(-num / max(den, 1.0))
    return np.asarray(loss, dtype=np.float32)


# revision 11
# speedup vs baseline: 1.0517x; 1.0517x over previous
"""Trainium2 Bass kernel for nn_Artificial_label_loss (retrieval_knn).

Shards across 8 NeuronCores: core c handles batch b=c//4 and query chunk
q=c%4 (2048 queries). One brute-force pass computes the (2048 x 8192) L1
distance tile set; row-mins give cham_x, row argmins come from max_index
(value search), and column-mins (for cham_y) are extracted from the same
distance tiles via TensorEngine transposes + free-dim reductions, then
min-combined across the 4 cores of the batch group with a ReduceScatter
that lands exactly this core's cham_y chunk. The epilogue (flow-vs-rigid
select, gather, grid scatter, cross-entropy partial sums) runs on-device
with an AllGather of (cell, label); the host combines two scalar sums.
"""
import os
import numpy as np

from concourse import bass, tile, mybir, bacc
from concourse.bass_utils import run_bass_kernel_spmd
from concourse.masks import make_identity

dt = mybir.dt
Alu = mybir.AluOpType
Act = mybir.ActivationFunctionType
AX = mybir.AxisListType

B, N, M, G = 2, 8192, 8192, 256
X_MIN = -35.0
CELL = abs(2.0 * X_MIN / G)          # 0.2734375, exact in f32
INV_CELL = np.float32(1.0) / np.float32(CELL)

P = 128          # partitions
NQT = 16         # query tiles per core (16*128 = 2048)
CH = 2048        # per-core chunk size
MT = 2048        # M tile size
NMT = M // MT    # 4
NBLK = MT // P   # 16 transpose blocks per M tile
GRP = 8          # transpose blocks per PSUM reduction group

NCORES = 8
RGROUPS = [[0, 1, 2, 3], [4, 5, 6, 7]]


def _build():
    nc = bacc.Bacc("TRN2", target_bir_lowering=False, debug=False,
                   num_devices=NCORES)

    # ---- inputs (per-core shards prepared by host) ----
    pjT = nc.dram_tensor("pjT", [3, M], dt.float32, kind="ExternalInput")
    piqT = nc.dram_tensor("piqT", [3, CH], dt.float32, kind="ExternalInput")
    pj = nc.dram_tensor("pj", [M, 3], dt.float32, kind="ExternalInput")
    flow = nc.dram_tensor("flow", [P, NQT], dt.float32, kind="ExternalInput")
    nf = nc.dram_tensor("nf", [P, NQT], dt.int32, kind="ExternalInput")
    mos0 = nc.dram_tensor("mos0", [P, 512], dt.float32, kind="ExternalInput")
    mos1 = nc.dram_tensor("mos1", [P, 512], dt.float32, kind="ExternalInput")

    o_sums = nc.dram_tensor("o_sums", [P, 2], dt.float32, kind="ExternalOutput")
    o_chamx = nc.dram_tensor("o_chamx", [P, NQT], dt.float32, kind="ExternalOutput")
    o_chamy = nc.dram_tensor("o_chamy", [P, NQT], dt.float32, kind="ExternalOutput")
    o_jstar = nc.dram_tensor("o_jstar", [P, NQT], dt.float32, kind="ExternalOutput")

    def bcast_ap(dram_t, coord, lo, n):
        return bass.AP(tensor=dram_t[:].tensor, offset=coord * dram_t.shape[1] + lo,
                       ap=[[0, P], [1, n]])

    with tile.TileContext(nc) as tc:
        with tc.tile_pool(name="persist", bufs=1) as pp:
            chamx = pp.tile([P, NQT], dt.float32)
            chamy = pp.tile([P, NQT], dt.float32)
            jstar = pp.tile([P, NQT], dt.float32)
            ident = pp.tile([P, P], dt.float32)
            make_identity(nc, ident[:])
            colmin = pp.tile([P, M // P], dt.float32)       # [128, 64]
            nc.vector.memset(colmin[:], 3.0e38)
            consts_i = pp.tile([P, NMT], dt.int32)
            consts = pp.tile([P, NMT], dt.float32)
            nc.gpsimd.iota(consts_i[:], pattern=[[MT, NMT]], base=0,
                           channel_multiplier=0)             # 0,2048,4096,6144
            nc.vector.tensor_copy(consts[:], consts_i[:])

            # ---------------- distance pass ----------------
            with tc.tile_pool(name="p1c", bufs=1) as cp, \
                 tc.tile_pool(name="p1d", bufs=7) as dp, \
                 tc.tile_pool(name="p1t", bufs=2) as tp, \
                 tc.tile_pool(name="p1s", bufs=2) as sp, \
                 tc.tile_pool(name="p1i", bufs=6) as ip, \
                 tc.tile_pool(name="psum", bufs=4, space="PSUM") as psp:
                tgt = []
                for c in range(3):
                    row = []
                    for m in range(NMT):
                        t = cp.tile([P, MT], dt.float32, name=f"tj{c}_{m}")
                        nc.sync.dma_start(t[:], bcast_ap(pjT, c, m * MT, MT))
                        row.append(t)
                    tgt.append(row)

                negq_all = cp.tile([P, NQT, 3], dt.float32)
                for c in range(3):
                    nc.sync.dma_start(
                        bass.AP(tensor=negq_all[:].tensor,
                                offset=negq_all[:].offset + c,
                                ap=[[NQT * 3, P], [3, NQT]]),
                        bass.AP(tensor=piqT[:].tensor, offset=c * CH,
                                ap=[[1, P], [P, NQT]]))
                nc.vector.tensor_scalar(negq_all[:], negq_all[:], -1.0, None,
                                        Alu.mult)

                for k in range(NQT):
                    negq = negq_all[:, k]
                    minacc = sp.tile([P, NMT], dt.float32, tag="minacc")
                    dms = []
                    for m in range(NMT):
                        dm = dp.tile([P, MT], dt.float32, tag="d", name=f"d_{k}_{m}")
                        dms.append(dm)
                        dx = tp.tile([P, MT], dt.float32, tag="dx")
                        dy = tp.tile([P, MT], dt.float32, tag="dy")
                        nc.scalar.activation(dm[:], tgt[2][m][:], Act.Abs,
                                             bias=negq[:, 2:3], scale=1.0)
                        nc.scalar.activation(dx[:], tgt[0][m][:], Act.Abs,
                                             bias=negq[:, 0:1], scale=1.0)
                        nc.scalar.activation(dy[:], tgt[1][m][:], Act.Abs,
                                             bias=negq[:, 1:2], scale=1.0)
                        nc.vector.tensor_tensor(out=dx[:], in0=dx[:], in1=dy[:],
                                                op=Alu.add)
                        nc.gpsimd.tensor_tensor(out=dm[:], in0=dx[:], in1=dm[:],
                                                op=Alu.add)
                        nc.vector.tensor_reduce(minacc[:, m:m + 1], dm[:],
                                                axis=AX.X, op=Alu.min)
                        # column mins via PE transpose + PSUM reduction
                        for g in range(NBLK // GRP):
                            ps = psp.tile([P, GRP * P], dt.float32, tag="ps")
                            for blk in range(GRP):
                                j0 = (g * GRP + blk) * P
                                nc.tensor.transpose(
                                    out=ps[:, blk * P:(blk + 1) * P],
                                    in_=dm[:, j0:j0 + P], identity=ident[:])
                            cm8 = sp.tile([P, GRP], dt.float32, tag="cm8")
                            nc.vector.tensor_reduce(
                                cm8[:], ps[:].rearrange("p (b j) -> p b j", b=GRP),
                                axis=AX.X, op=Alu.min)
                            csl = colmin[:, m * NBLK + g * GRP:
                                         m * NBLK + (g + 1) * GRP]
                            nc.vector.tensor_tensor(out=csl, in0=csl, in1=cm8[:],
                                                    op=Alu.min)
                    nc.vector.tensor_reduce(chamx[:, k:k + 1], minacc[:],
                                            axis=AX.X, op=Alu.min)
                    # row argmin: search the min value in each d tile
                    minv8 = sp.tile([P, 8], dt.float32, tag="minv8")
                    nc.vector.tensor_copy(minv8[:],
                                          chamx[:, k:k + 1].to_broadcast([P, 8]))
                    jg = sp.tile([P, NMT], dt.float32, tag="jg")
                    for m in range(NMT):
                        idx8 = ip.tile([P, 8], dt.uint32, tag="idx8")
                        nc.vector.max_index(idx8[:], minv8[:], dms[m][:])
                        nc.vector.tensor_copy(jg[:, m:m + 1], idx8[:, 0:1])
                    nc.vector.tensor_tensor(out=jg[:], in0=jg[:], in1=consts[:],
                                            op=Alu.add)
                    nc.vector.tensor_reduce(jstar[:, k:k + 1], jg[:],
                                            axis=AX.X, op=Alu.min)

            # ---------------- cham_y via ReduceScatter(min) ----------------
            with tc.tile_pool(name="ep", bufs=1) as ep, \
                 tc.tile_pool(name="epd", bufs=1, space="DRAM") as epd:
                rs_in = epd.tile([M // P, P], dt.float32)     # [64, 128]
                rs_out = epd.tile([M // P // 4, P], dt.float32)  # [16, 128]
                nc.sync.dma_start(
                    bass.AP(tensor=rs_in[:].tensor, offset=rs_in[:].offset,
                            ap=[[1, P], [P, M // P]]), colmin[:])
                nc.gpsimd.collective_compute(
                    "ReduceScatter", Alu.min, replica_groups=RGROUPS,
                    ins=[rs_in[:].opt()], outs=[rs_out[:].opt()])
                nc.sync.dma_start(
                    chamy[:],
                    bass.AP(tensor=rs_out[:].tensor, offset=rs_out[:].offset,
                            ap=[[1, P], [P, NQT]]))

                # ---------------- epilogue ----------------
                nc.sync.dma_start(o_chamx[:], chamx[:])
                nc.sync.dma_start(o_chamy[:], chamy[:])
                nc.sync.dma_start(o_jstar[:], jstar[:])

                flw = ep.tile([P, NQT], dt.float32)
                nc.sync.dma_start(flw[:], flow[:])
                nff = ep.tile([P, NQT], dt.int32)
                nc.sync.dma_start(nff[:], nf[:])
                nff_f = ep.tile([P, NQT], dt.float32)
                nc.vector.tensor_copy(nff_f[:], nff[:])

                rigid = ep.tile([P, NQT], dt.float32)
                nc.vector.tensor_tensor(out=rigid[:], in0=chamx[:], in1=chamy[:],
                                        op=Alu.add)
                nc.vector.tensor_scalar(rigid[:], rigid[:], 0.5, None, Alu.mult)
                dyn = ep.tile([P, NQT], dt.float32)
                nc.vector.tensor_tensor(out=dyn[:], in0=flw[:], in1=rigid[:],
                                        op=Alu.is_gt)
                labels = ep.tile([P, NQT], dt.int32)
                nc.vector.tensor_copy(labels[:], dyn[:])

                # idx = jstar + dyn * (nf - jstar)
                idxf = ep.tile([P, NQT], dt.float32)
                nc.vector.tensor_tensor(out=idxf[:], in0=nff_f[:], in1=jstar[:],
                                        op=Alu.subtract)
                nc.vector.tensor_tensor(out=idxf[:], in0=idxf[:], in1=dyn[:],
                                        op=Alu.mult)
                nc.vector.tensor_tensor(out=idxf[:], in0=idxf[:], in1=jstar[:],
                                        op=Alu.add)
                idxi = ep.tile([P, NQT], dt.int32)
                nc.vector.tensor_copy(idxi[:], idxf[:])

                gxyz = ep.tile([P, NQT, 3], dt.float32)
                for k in range(NQT):
                    nc.gpsimd.indirect_dma_start(
                        out=gxyz[:, k, :], out_offset=None, in_=pj[:],
                        in_offset=bass.IndirectOffsetOnAxis(ap=idxi[:, k:k + 1],
                                                            axis=0))

                # cell indices (neuron-backend astype rounds to nearest)
                cellx = ep.tile([P, NQT], dt.float32)
                celly = ep.tile([P, NQT], dt.float32)
                nc.vector.tensor_scalar(cellx[:], gxyz[:, :, 0], -X_MIN,
                                        float(INV_CELL), Alu.add, Alu.mult)
                nc.vector.tensor_scalar(celly[:], gxyz[:, :, 1], -X_MIN,
                                        float(INV_CELL), Alu.add, Alu.mult)
                cxi = ep.tile([P, NQT], dt.int32)
                cyi = ep.tile([P, NQT], dt.int32)
                nc.vector.tensor_copy(cxi[:], cellx[:])
                nc.vector.tensor_copy(cyi[:], celly[:])
                cells = ep.tile([P, NQT], dt.int32)
                nc.vector.tensor_scalar(cells[:], cxi[:], G, None, Alu.mult)
                nc.vector.tensor_tensor(out=cells[:], in0=cells[:], in1=cyi[:],
                                        op=Alu.add)

                # per-core grid scatter of OWN chunk (query order = last wins),
                # then AllGather the 4 partial grids and merge by chunk priority
                grid_d = epd.tile([G * G, 1], dt.int32)
                initm = ep.tile([P, 512], dt.int32)
                nc.vector.memset(initm[:], -1)
                nc.sync.dma_start(
                    bass.AP(tensor=grid_d[:].tensor, offset=grid_d[:].offset,
                            ap=[[512, P], [1, 512]]), initm[:])
                for col in range(NQT):
                    nc.gpsimd.indirect_dma_start(
                        out=grid_d[:],
                        out_offset=bass.IndirectOffsetOnAxis(
                            ap=cells[:, col:col + 1], axis=0),
                        in_=labels[:, col:col + 1], in_offset=None)
                gag = epd.tile([4, G * G], dt.int32)
                nc.gpsimd.collective_compute(
                    "AllGather", Alu.bypass, replica_groups=RGROUPS,
                    ins=[bass.AP(tensor=grid_d[:].tensor,
                                 offset=grid_d[:].offset,
                                 ap=[[G * G, 1], [1, G * G]]).opt()],
                    outs=[gag[:].opt()])
                # merge: start from chunk 3, fill -1 holes from earlier chunks
                grid = ep.tile([P, 512], dt.int32)
                gtmp = ep.tile([P, 512], dt.int32)
                hole = ep.tile([P, 512], dt.int32)
                for gI in range(3, -1, -1):
                    gap = bass.AP(tensor=gag[:].tensor,
                                  offset=gag[:].offset + gI * G * G,
                                  ap=[[512, P], [1, 512]])
                    if gI == 3:
                        nc.sync.dma_start(grid[:], gap)
                    else:
                        nc.sync.dma_start(gtmp[:], gap)
                        nc.vector.tensor_scalar(hole[:], grid[:], 0.0, None,
                                                Alu.is_lt)
                        nc.vector.copy_predicated(grid[:], hole[:], gtmp[:])

                # CE partial sums
                m0 = ep.tile([P, 512], dt.float32)
                m1 = ep.tile([P, 512], dt.float32)
                nc.sync.dma_start(m0[:], mos0[:])
                nc.sync.dma_start(m1[:], mos1[:])
                e0 = ep.tile([P, 512], dt.float32)
                e1 = ep.tile([P, 512], dt.float32)
                nc.scalar.activation(e0[:], m0[:], Act.Exp)
                nc.scalar.activation(e1[:], m1[:], Act.Exp)
                nc.vector.tensor_tensor(out=e0[:], in0=e0[:], in1=e1[:], op=Alu.add)
                lse = ep.tile([P, 512], dt.float32)
                nc.scalar.activation(lse[:], e0[:], Act.Ln)
                lp0 = ep.tile([P, 512], dt.float32)
                lp1 = ep.tile([P, 512], dt.float32)
                nc.vector.tensor_tensor(out=lp0[:], in0=m0[:], in1=lse[:],
                                        op=Alu.subtract)
                nc.vector.tensor_tensor(out=lp1[:], in0=m1[:], in1=lse[:],
                                        op=Alu.subtract)
                valid = ep.tile([P, 512], dt.float32)
                nc.vector.tensor_scalar(valid[:], grid[:], 0.0, None, Alu.is_ge)
                tsel = ep.tile([P, 512], dt.float32)
                nc.vector.tensor_scalar(tsel[:], grid[:], 0.0, None, Alu.max)
                nc.vector.tensor_tensor(out=lp1[:], in0=lp1[:], in1=lp0[:],
                                        op=Alu.subtract)
                nc.vector.tensor_tensor(out=lp1[:], in0=lp1[:], in1=tsel[:],
                                        op=Alu.mult)
                nc.vector.tensor_tensor(out=lp1[:], in0=lp1[:], in1=lp0[:],
                                        op=Alu.add)
                nc.vector.tensor_tensor(out=lp1[:], in0=lp1[:], in1=valid[:],
                                        op=Alu.mult)
                sums = ep.tile([P, 2], dt.float32)
                nc.vector.tensor_reduce(sums[:, 0:1], lp1[:], axis=AX.X,
                                        op=Alu.add)
                nc.vector.tensor_reduce(sums[:, 1:2], valid[:], axis=AX.X,
                                        op=Alu.add)
                nc.sync.dma_start(o_sums[:], sums[:])

    nc.compile()
    return nc


_NC = None


def _get_nc():
    global _NC
    if _NC is None:
        _NC = _build()
    return _NC


_LAST_RESULTS = None


def kernel(p_i, mos, p_j, error_p_i_flow, nearest_flow):
    global _LAST_RESULTS
    p_i = np.ascontiguousarray(np.asarray(p_i, np.float32))
    p_j = np.ascontiguousarray(np.asarray(p_j, np.float32))
    mos = np.asarray(mos, np.float32)
    flow = np.asarray(error_p_i_flow, np.float32)
    nf = np.asarray(nearest_flow).astype(np.int32)

    nc = _get_nc()
    in_maps = []
    for c in range(NCORES):
        b, q = divmod(c, 4)
        s = q * CH
        in_maps.append({
            "pjT": np.ascontiguousarray(p_j[b].T),
            "piqT": np.ascontiguousarray(p_i[b, s:s + CH].T),
            "pj": p_j[b],
            "flow": np.ascontiguousarray(flow[b, s:s + CH].reshape(NQT, P).T),
            "nf": np.ascontiguousarray(nf[b, s:s + CH, 0].reshape(NQT, P).T),
            "mos0": np.ascontiguousarray(mos[b, 0].reshape(P, 512)),
            "mos1": np.ascontiguousarray(mos[b, 1].reshape(P, 512)),
        })

    trace = bool(int(os.environ.get("KNN_TRACE", "0")))
    tmpdir = os.environ.get("KNN_TMPDIR") or None
    res = run_bass_kernel_spmd(nc, in_maps, core_ids=list(range(NCORES)),
                               trace=trace, tmpdir=tmpdir)
    _LAST_RESULTS = res

    s0 = res.results[0]["o_sums"].astype(np.float64)
    s1 = res.results[4]["o_sums"].astype(np.float64)
    num = np.float32(s0[:, 0].sum() + s1[:, 0].sum())
    den = np.float32(s0[:, 1].sum() + s1[:, 1].sum())
    loss = np.float32(-num / max(den, 1.0))
    return np.asarray(loss, dtype=np.float32)
